# revision 25
# baseline (speedup 1.0000x reference)
"""Trainium2 Bass kernel for nn_BasicRNN_5909874999704.

RNN: B=64, T=512, D=256, H=1024, O=256
  per step: pre = x_t @ W_in.T + b_in + h @ W_h.T + b_h
            h'  = 0.5*relu(pre) + 0.5*h
            out = h' @ W_out.T + b_out
  returns (outs [B,T,O], hs [B,T,H])

Strategy (8 NeuronCores, data-parallel over batch, B_local=8 per core):
  Only the W_h @ h matvec is inherently serial; we phase-split:
    phase 1 (parallel over t): u = 0.5*(W_in x_t + b_in + b_h)  [big matmuls]
    phase 2 (serial over t):   f_{t+1} = 0.5*f_t + relu(0.5*psum_t),
                               psum_t = W_h f_t + u_t  where f = h/2 (bf16)
    phase 3 (parallel over t): outs = 2*(W_out f) + b_out; hs = 2*f (PE transpose)
  Phase-2 layout: f is [128 part, 8 hchunk x 8 batch] folded; W_h^T chunks are
  the PE stationary operand (bf16, fast weight load); per step 64 matmuls of
  [128,128]x[128,8] accumulate into one PSUM bank plus one fp32 identity-matmul
  that injects u_t. Work proceeds in 64-step windows entirely out of SBUF.
  Host pre-transposes weights/activations so every DMA is a contiguous
  natural tiled load.
"""

import sys
from contextlib import ExitStack

import numpy as np

for _p in ("/opt/trn_rl_repo", "/opt/pypackages"):
    if _p not in sys.path:
        sys.path.insert(0, _p)

import concourse.bacc as bacc
import concourse.mybir as mybir
import concourse.tile as tile
from concourse import bass_utils
from concourse.masks import make_identity

F32 = mybir.dt.float32
F32R = mybir.dt.float32r
BF16 = mybir.dt.bfloat16
AF = mybir.ActivationFunctionType
ALU = mybir.AluOpType

B, T, D, H, O = 64, 512, 256, 1024, 256
NCORES = 8
BL = B // NCORES  # 8 batch rows per core
KC = H // 128     # 8 hidden chunks
DC = D // 128     # 2 input chunks
WIN = 64          # steps per window


def build(nc_T=T, win=WIN, hist_f32=False, p1_f32=False, r_f32=True,
          u_bf16=False, p1_f32r=True, u_f32r=True):
    """Build the per-core Bass program (SPMD, same program all cores)."""
    nw = nc_T // win
    assert nw * win == nc_T
    HDT = F32 if hist_f32 else BF16
    XDT = F32R if p1_f32r else (F32 if p1_f32 else BF16)
    RDT = F32 if (r_f32 or hist_f32) else BF16
    UDT = F32R if u_f32r else (BF16 if u_bf16 else F32)
    nc = bacc.Bacc("TRN2", target_bir_lowering=False, debug=False,
                   num_devices=NCORES)

    obst = nc.dram_tensor("obsT", [D, nc_T, BL], F32, kind="ExternalInput").ap()
    h0h = nc.dram_tensor("h0half", [H, BL], F32, kind="ExternalInput").ap()
    w_ht = nc.dram_tensor("W_hT", [H, H], F32, kind="ExternalInput").ap()
    w_int = nc.dram_tensor("W_inT", [D, H], F32, kind="ExternalInput").ap()
    w_outt = nc.dram_tensor("W_outT", [H, O], F32, kind="ExternalInput").ap()
    biasmp = nc.dram_tensor("biasmp", [128, KC], F32, kind="ExternalInput").ap()
    b_oh = nc.dram_tensor("b_out_half", [1, O], F32, kind="ExternalInput").ap()
    outs = nc.dram_tensor("outs", [BL, nc_T, O], F32, kind="ExternalOutput").ap()
    hs = nc.dram_tensor("hs", [BL, nc_T, H], F32, kind="ExternalOutput").ap()

    with tile.TileContext(nc) as tc, ExitStack() as ctx:
        const = ctx.enter_context(tc.tile_pool(name="const", bufs=1))
        hist_p = ctx.enter_context(tc.tile_pool(name="hist", bufs=2))
        u_p = ctx.enter_context(tc.tile_pool(name="u", bufs=2))
        x_p = ctx.enter_context(tc.tile_pool(name="x", bufs=2))
        r_p = ctx.enter_context(tc.tile_pool(name="r", bufs=2))
        st_p = ctx.enter_context(tc.tile_pool(name="st", bufs=2))
        ps2_p = ctx.enter_context(tc.tile_pool(name="ps2", bufs=2, space="PSUM"))
        ps1_p = ctx.enter_context(tc.tile_pool(name="ps1", bufs=2, space="PSUM"))
        ps3_p = ctx.enter_context(tc.tile_pool(name="ps3", bufs=2, space="PSUM"))
        pstr_p = ctx.enter_context(tc.tile_pool(name="pstr", bufs=2, space="PSUM"))

        # ---- constants / weights in SBUF ----
        w2t = const.tile([128, KC * KC * 128], HDT, tag="w2t")
        nc.gpsimd.dma_start(
            out=w2t[:].rearrange("p (k r) -> p k r", k=KC),
            in_=w_ht.rearrange("(k p) r -> p k r", p=128))
        wit = const.tile([128, DC * KC * 128], XDT, tag="wit")
        nc.gpsimd.dma_start(
            out=wit[:].rearrange("p (k r) -> p k r", k=DC),
            in_=w_int.rearrange("(k p) r -> p k r", p=128))
        wot = const.tile([128, KC * O], HDT, tag="wot")
        nc.gpsimd.dma_start(
            out=wot[:].rearrange("p (k o) -> p k o", k=KC),
            in_=w_outt.rearrange("(k p) o -> p k o", p=128))

        bias = const.tile([128, KC], F32, tag="bias")
        nc.sync.dma_start(out=bias[:], in_=biasmp)
        bo_half = const.tile([1, O], BF16, tag="bo_half")
        nc.gpsimd.dma_start(out=bo_half[:], in_=b_oh)
        ones = const.tile([1, 128], BF16, tag="ones")
        nc.vector.memset(ones[:], 1.0)
        ones_f = const.tile([1, 128], F32, tag="ones_f")
        nc.vector.memset(ones_f[:], 1.0)
        bo_half_f = const.tile([1, O], F32, tag="bo_half_f")
        nc.sync.dma_start(out=bo_half_f[:], in_=b_oh)

        ident = const.tile([128, 128], F32, tag="ident")
        make_identity(nc, ident[:])
        ident_bf = const.tile([128, 128], BF16, tag="ident_bf")
        nc.vector.tensor_copy(ident_bf[:], ident[:])
        ident_h = ident if hist_f32 else ident_bf
        ident_r = const.tile([128, 128], F32R, tag="ident_r")
        nc.vector.tensor_copy(ident_r[:], ident[:])

        # ---- state ----
        # hist: f = h/2; col layout (k, slot, b): slot s of chunk k at
        # cols k*(win+1)*BL + s*BL + b. Slot s holds state at start of step s.
        # Double-buffered per window so phase3(w)/phase1(w+1) overlap
        # phase2(w+1).
        nslot = win + 1

        def w2t_tile(k, m):
            i = (k * KC + m) * 128
            return w2t[:, i:i + 128]

        prev_hist3 = None
        for w in range(nw):
            t0 = w * win
            hist = hist_p.tile([128, KC * nslot * BL], HDT, tag="hist")
            u_sb = u_p.tile([128, win * 64], UDT, tag="u_sb")
            hist3 = hist[:].rearrange("p (k s b) -> p k s b", k=KC, s=nslot)

            def hist_kslot(k, s, ns=1, hist=hist):
                base = k * nslot * BL + s * BL
                return hist[:, base:base + ns * BL]

            if w == 0:
                nc.gpsimd.dma_start(
                    out=hist3[:, :, 0, :],
                    in_=h0h.rearrange("(k p) b -> p k b", p=128))
            else:
                nc.vector.tensor_copy(hist3[:, :, 0, :],
                                      prev_hist3[:, :, win, :])
            prev_hist3 = hist3
            # ---------- phase 1: u = 0.5*(W_in x + b_in + b_h) ----------
            xw = x_p.tile([128, DC * win * BL], XDT, tag="xw")
            nc.gpsimd.dma_start(
                out=xw[:].rearrange("p (k c) -> p k c", k=DC),
                in_=obst[:, t0:t0 + win, :].rearrange(
                    "(k p) t b -> p k (t b)", p=128))
            u3 = u_sb[:].rearrange("p (t k b) -> p t (k b)", k=KC, b=BL)
            for m in range(KC):
                ps = ps1_p.tile([128, win * BL], F32, tag="ps1")
                for k2 in range(DC):
                    nc.tensor.matmul(
                        ps[:],
                        lhsT=wit[:, (k2 * KC + m) * 128:(k2 * KC + m + 1) * 128],
                        rhs=xw[:, k2 * win * BL:(k2 + 1) * win * BL],
                        start=(k2 == 0), stop=(k2 == DC - 1))
                # u[:, t, m*8+b] = (ps + bias[m]) * 0.5
                nc.vector.tensor_scalar(
                    out=u3[:, :, m * BL:(m + 1) * BL],
                    in0=ps[:].rearrange("p (t b) -> p t b", b=BL),
                    scalar1=bias[:, m:m + 1],
                    scalar2=0.5,
                    op0=ALU.add, op1=ALU.mult)

            # ---------- phase 2: serial recurrence ----------
            for tl in range(win):
                ps = ps2_p.tile([128, 64], F32, tag="ps2")
                nc.tensor.matmul(ps[:], lhsT=(ident_r[:] if u_f32r else (ident_bf[:] if u_bf16 else ident[:])),
                                 rhs=u_sb[:, tl * 64:(tl + 1) * 64],
                                 start=True, stop=False)
                for m in range(KC):
                    for k in range(KC):
                        nc.tensor.matmul(
                            ps[:, m * BL:(m + 1) * BL],
                            lhsT=w2t_tile(k, m),
                            rhs=hist_kslot(k, tl),
                            start=False,
                            stop=(m == KC - 1 and k == KC - 1))
                r = r_p.tile([128, 64], RDT, tag="r")
                nc.scalar.activation(r[:], ps[:], AF.Relu, bias=0.0, scale=0.5)
                # f_{t+1} = 0.5*f_t + r
                nc.vector.scalar_tensor_tensor(
                    out=hist3[:, :, tl + 1, :], in0=hist3[:, :, tl, :],
                    scalar=0.5,
                    in1=r[:].rearrange("p (m b) -> p m b", b=BL),
                    op0=ALU.mult, op1=ALU.add)

            # ---------- phase 3: outs + hs for this window ----------
            nchunk = (win * BL) // 128  # 128-row (t,b) chunks
            tpc = 128 // BL             # timesteps per chunk
            for c in range(nchunk):
                s_lo = 1 + c * tpc
                ts_lo = t0 + c * tpc
                ps = ps3_p.tile([128, O], F32, tag="ps3")
                for k in range(KC):
                    nc.tensor.matmul(
                        ps[:],
                        lhsT=hist_kslot(k, s_lo, ns=tpc),
                        rhs=wot[:, k * O:(k + 1) * O],
                        start=(k == 0), stop=False)
                nc.tensor.matmul(ps[:], lhsT=(ones_f[:] if hist_f32 else ones[:]),
                                 rhs=(bo_half_f[:] if hist_f32 else bo_half[:]),
                                 start=False, stop=True)
                ost = st_p.tile([128, O], F32, tag="ost")
                nc.scalar.activation(ost[:], ps[:], AF.Copy, bias=0.0, scale=2.0)
                nc.sync.dma_start(
                    out=outs[:, ts_lo:ts_lo + tpc, :].rearrange("b t o -> t b o"),
                    in_=ost[:])

                hst = st_p.tile([128, H], F32, tag="hst")
                for j in range(KC):
                    pst = pstr_p.tile([128, 128], HDT, tag="pstr")
                    nc.tensor.transpose(pst[:], in_=hist_kslot(j, s_lo, ns=tpc),
                                        identity=ident_h[:])
                    nc.scalar.activation(hst[:, j * 128:(j + 1) * 128], pst[:],
                                         AF.Copy, bias=0.0, scale=2.0)
                nc.sync.dma_start(
                    out=hs[:, ts_lo:ts_lo + tpc, :].rearrange("b t h -> t b h"),
                    in_=hst[:])

    nc.compile()
    return nc


_CACHE = {}


def _get_nc(nc_T=T, win=WIN, **kw):
    key = (nc_T, win, tuple(sorted(kw.items())))
    if key not in _CACHE:
        _CACHE[key] = build(nc_T, win, **kw)
    return _CACHE[key]


def prep_inputs(inputs, nc_T=T):
    """Host-side sharding + layout transforms."""
    obs = np.ascontiguousarray(np.asarray(inputs["obs_seq"], dtype=np.float32))
    h0 = np.ascontiguousarray(np.asarray(inputs["h0"], dtype=np.float32))
    w_in = np.asarray(inputs["W_in"], dtype=np.float32)
    b_in = np.asarray(inputs["b_in"], dtype=np.float32)
    w_h = np.asarray(inputs["W_h"], dtype=np.float32)
    b_h = np.asarray(inputs["b_h"], dtype=np.float32)
    w_out = np.asarray(inputs["W_out"], dtype=np.float32)
    b_out = np.asarray(inputs["b_out"], dtype=np.float32)

    shared = {
        "W_hT": np.ascontiguousarray(w_h.T),
        "W_inT": np.ascontiguousarray(w_in.T),
        "W_outT": np.ascontiguousarray(w_out.T),
        "biasmp": np.ascontiguousarray((b_in + b_h).reshape(KC, 128).T),
        "b_out_half": np.ascontiguousarray((0.5 * b_out).reshape(1, O)),
    }
    in_maps = []
    for c in range(NCORES):
        m = dict(shared)
        m["obsT"] = np.ascontiguousarray(
            obs[c * BL:(c + 1) * BL, :nc_T].transpose(2, 1, 0))
        m["h0half"] = np.ascontiguousarray(0.5 * h0[c * BL:(c + 1) * BL].T)
        in_maps.append(m)
    return in_maps


def run_on_cores(inputs, nc_T=T, win=WIN, trace=False, **kw):
    nc = _get_nc(nc_T, win, **kw)
    in_maps = prep_inputs(inputs, nc_T)
    res = bass_utils.run_bass_kernel_spmd(nc, in_maps, list(range(NCORES)),
                                          trace=trace)
    outs = np.concatenate([r["outs"] for r in res.results], axis=0)
    hss = np.concatenate([r["hs"] for r in res.results], axis=0)
    return (outs, hss), res


def kernel(**inputs):
    (outs, hss), _ = run_on_cores(inputs)
    return outs, hss


if __name__ == "__main__":
    import time
    t0 = time.time()
    nc = _get_nc()
    print(f"build took {time.time() - t0:.1f}s")
